# revision 14
# baseline (speedup 1.0000x reference)
"""Trainium2 Bass kernel: per-token int8 fake-quant x  @  int4-group-dequant W^T.

Math (matches torchao-style reference):
    x_dq = per_token_quant_dequant(x)            # [B*S, I]
    w_dq = (w_int - zeros) * scales per group    # [O, I]
    out  = x_dq @ w_dq.T                         # [B*S, O]

Key factorization used on device:
    x_dq[t, i] = s[t] * qmz[t, i]   with qmz integer in [-255, 255]  (exact in bf16)
    out[t, o]  = s[t] * sum_i qmz[t, i] * w_dq[o, i]
The integer-valued qmz matmuls exactly; w_dq is split hi+lo in bf16 so the
weight product is exact to ~2^-17. Per-token scale applied to output rows.

Sharding: data-parallel over tokens, 8 cores x 1024 tokens each.
Each core dequantizes the full weight (overlapped with PE work).
"""

import math
import os
from contextlib import ExitStack

import numpy as np

import concourse.bass as bass
import concourse.mybir as mybir
import concourse.tile as tile
from concourse import bass_utils
from concourse.vector_clock import ScopedClock
from concourse import masks

FP = mybir.dt.float32
BF = mybir.dt.bfloat16
F16 = mybir.dt.float16
I8 = mybir.dt.int8
ALU = mybir.AluOpType
ACTF = mybir.ActivationFunctionType

MAGIC = 12582912.0  # 1.5 * 2**23: add/sub forces RNE round-to-integer in fp32
EPS32 = float(np.finfo(np.float32).eps)
QMAX = 127.0
GROUP = 32

N_CORES = 8
B, S, D_IN, D_OUT = 4, 2048, 2048, 2048
TOK_FULL = B * S


SplitDrainTileContext = tile.TileContext

MAX_WAITS_PER_INST = 1


def split_excess_waits(nc, max_waits=MAX_WAITS_PER_INST):
    """This walrus build rejects instructions with more than a couple of
    sync-wait commands. Move excess waits onto same-engine NOPs placed
    immediately before the over-subscribed instruction — semantically
    identical (the engine performs all waits before issuing)."""
    n_split = 0
    for f in nc.m.functions:
        for bb in f.blocks:
            insts = bb.instructions
            if not any(
                i.sync_info is not None and len(i.sync_info.on_wait or []) > max_waits
                for i in insts
            ):
                continue
            new = []
            for inst in insts:
                si = inst.sync_info
                waits = list(si.on_wait) if si is not None and si.on_wait else []
                if len(waits) > max_waits:
                    keep = waits[-max_waits:]
                    rest = waits[: len(waits) - max_waits]
                    for j in range(0, len(rest), max_waits):
                        nop = mybir.InstNoOp(
                            name=f"wsplit_{inst.name}_{j}",
                            engine=inst.engine,
                            ins=[],
                            outs=[],
                            sync_info=mybir.SyncInfo(
                                on_wait=rest[j : j + max_waits], on_update=[]
                            ),
                        )
                        new.append(nop)
                        n_split += 1
                    si.on_wait = keep
                new.append(inst)
            insts[:] = new
    return n_split


def build_nc(tok, d_in, d_out, oq=512, two_pass=True, wdt=BF, split_waits=True):
    """Build the per-core program.

    tok:   tokens handled by this core
    d_in:  contraction dim (I), multiple of 128
    d_out: output features (O), multiple of oq
    oq:    output-quarter width processed per weight-residency round
    two_pass: hi+lo exact weight split (True) or single rounded pass
    wdt:   matmul dtype for weights/activations (bf16 or fp16)
    """
    nt = tok // 128      # token tiles
    ni = d_in // 128     # contraction tiles
    nq = d_out // oq     # weight residency rounds
    noc = oq // 512      # psum chunks per round
    assert oq % 512 == 0 and tok % 128 == 0 and d_in % 128 == 0 and d_out % oq == 0
    g_per_i = 128 // GROUP  # scale rows covered by one contraction tile

    nc = bass.Bass("TRN2", target_bir_lowering=False, debug=False)
    xs = nc.dram_tensor("xs", [tok, d_in], FP, kind="ExternalInput").ap()
    w8t = nc.dram_tensor("w8t", [d_in, d_out], I8, kind="ExternalInput").ap()
    st = nc.dram_tensor("st", [d_in // GROUP, d_out], FP, kind="ExternalInput").ap()
    out = nc.dram_tensor("out", [tok, d_out], FP, kind="ExternalOutput").ap()

    with SplitDrainTileContext(nc) as tc, ExitStack() as ctx:
        const_pool = ctx.enter_context(tc.tile_pool(name="const", bufs=1))
        ident = const_pool.tile([128, 128], wdt, tag="ident", name="ident")
        masks.make_identity(nc, ident[:])

        stats = ctx.enter_context(tc.tile_pool(name="stats", bufs=1))
        xp = ctx.enter_context(tc.tile_pool(name="xp", bufs=2))
        qa_p = ctx.enter_context(tc.tile_pool(name="qa", bufs=2))
        qb_p = ctx.enter_context(tc.tile_pool(name="qb", bufs=2))
        qc_p = ctx.enter_context(tc.tile_pool(name="qc", bufs=2))
        qxt_p = ctx.enter_context(tc.tile_pool(name="qxt", bufs=1))
        w8_p = ctx.enter_context(tc.tile_pool(name="w8", bufs=3))
        sc_p = ctx.enter_context(tc.tile_pool(name="sc", bufs=3))
        wf_p = ctx.enter_context(tc.tile_pool(name="wf", bufs=3))
        whi_p = ctx.enter_context(tc.tile_pool(name="whi", bufs=2))
        wlo_p = ctx.enter_context(tc.tile_pool(name="wlo", bufs=2))
        out_p = ctx.enter_context(tc.tile_pool(name="outp", bufs=4))
        ps_mm = ctx.enter_context(tc.tile_pool(name="psmm", bufs=4, space="PSUM"))
        ps_tr = ctx.enter_context(tc.tile_pool(name="pstr", bufs=3, space="PSUM"))

        # ---- phase A: per-token quant; qxT[i][:, t*128:...] = transposed int acts
        qxt = [qxt_p.tile([128, tok], wdt, tag=f"qxt{i}", name=f"qxt{i}") for i in range(ni)]
        s_tiles = []
        for t in range(nt):
            xt = xp.tile([128, d_in], FP)
            nc.sync.dma_start(xt[:], xs[t * 128 : (t + 1) * 128, :])

            mn = stats.tile([128, 1], FP, tag=f"mn{t}", name=f"mn{t}")
            mx = stats.tile([128, 1], FP, tag=f"mx{t}", name=f"mx{t}")
            nc.vector.tensor_reduce(mn[:], xt[:], mybir.AxisListType.X, ALU.min)
            nc.vector.tensor_reduce(mx[:], xt[:], mybir.AxisListType.X, ALU.max)
            # s = max((max(mx,0) - min(mn,0)) / 255, eps); inv = 1/s
            nc.vector.tensor_scalar(mn[:], mn[:], 0.0, None, ALU.min)
            nc.vector.tensor_scalar(mx[:], mx[:], 0.0, None, ALU.max)
            s_t = stats.tile([128, 1], FP, tag=f"s{t}", name=f"s{t}")
            nc.vector.tensor_tensor(s_t[:], mx[:], mn[:], ALU.subtract)
            nc.vector.tensor_scalar(
                s_t[:], s_t[:], float(np.float32(1.0) / np.float32(255.0)), EPS32,
                ALU.mult, ALU.max,
            )
            inv = stats.tile([128, 1], FP, tag=f"inv{t}", name=f"inv{t}")
            nc.vector.reciprocal(inv[:], s_t[:])
            # zp = -128 - rne(mn*inv);  biasB = zp - MAGIC
            u = stats.tile([128, 1], FP, tag=f"u{t}", name=f"u{t}")
            nc.vector.tensor_tensor(u[:], mn[:], inv[:], ALU.mult)
            nc.vector.tensor_scalar(u[:], u[:], MAGIC, None, ALU.add)
            nc.vector.tensor_scalar(u[:], u[:], MAGIC, None, ALU.subtract)
            zp = stats.tile([128, 1], FP, tag=f"zp{t}", name=f"zp{t}")
            nc.vector.tensor_scalar(zp[:], u[:], -1.0, -128.0, ALU.mult, ALU.add)
            biasb = stats.tile([128, 1], FP, tag=f"bb{t}", name=f"bb{t}")
            nc.vector.tensor_scalar(biasb[:], zp[:], MAGIC, None, ALU.subtract)
            s_tiles.append(s_t)

            # qmz = min(rne(x*inv) + zp, 127) - zp   (>= -128 guaranteed)
            qa = qa_p.tile([128, d_in], FP)
            nc.gpsimd.tensor_scalar(qa[:], xt[:], inv[:], MAGIC, ALU.mult, ALU.add)
            qb = qb_p.tile([128, d_in], FP)
            nc.scalar.activation(qb[:], qa[:], ACTF.Identity, bias=biasb[:], scale=1.0)
            qc = qc_p.tile([128, d_in], wdt)
            nc.vector.tensor_scalar(qc[:], qb[:], QMAX, zp[:], ALU.min, ALU.subtract)

            for i in range(ni):
                tr = ps_tr.tile([128, 128], wdt)
                nc.tensor.transpose(tr[:], qc[:, i * 128 : (i + 1) * 128], ident[:])
                nc.scalar.activation(
                    qxt[i][:, t * 128 : (t + 1) * 128], tr[:], ACTF.Copy
                )

        # ---- phases B+C interleaved per output quarter
        for q in range(nq):
            o0 = q * oq
            whi = []
            wlo = []
            for i in range(ni):
                w8 = w8_p.tile([128, oq], I8)
                nc.sync.dma_start(
                    w8[:], w8t[i * 128 : (i + 1) * 128, o0 : o0 + oq]
                )
                sc = sc_p.tile([128, oq], FP)
                for g in range(g_per_i):
                    src = st[i * g_per_i + g : i * g_per_i + g + 1, o0 : o0 + oq]
                    nc.sync.dma_start(
                        sc[g * GROUP : (g + 1) * GROUP, :],
                        src.partition_broadcast(GROUP),
                    )

                hi = whi_p.tile([128, oq], wdt, tag=f"whi{i}", name=f"whi_q{q}_{i}")
                if two_pass:
                    wf = wf_p.tile([128, oq], FP)
                    nc.vector.tensor_tensor(wf[:], w8[:], sc[:], ALU.mult)
                    nc.scalar.activation(hi[:], wf[:], ACTF.Copy)
                    lo = wlo_p.tile([128, oq], wdt, tag=f"wlo{i}", name=f"wlo_q{q}_{i}")
                    nc.vector.tensor_tensor(lo[:], wf[:], hi[:], ALU.subtract)
                    wlo.append(lo)
                else:
                    nc.vector.tensor_tensor(hi[:], w8[:], sc[:], ALU.mult)
                whi.append(hi)

            for t in range(nt):
                psums = [
                    ps_mm.tile([128, 512], FP, tag="ps", name=f"ps_q{q}_t{t}_{_oc}")
                    for _oc in range(noc)
                ]
                for i in range(ni):
                    lhs = qxt[i][:, t * 128 : (t + 1) * 128]
                    for oc in range(noc):
                        rhs_hi = whi[i][:, oc * 512 : (oc + 1) * 512]
                        nc.tensor.matmul(
                            psums[oc][:],
                            lhs,
                            rhs_hi,
                            start=(i == 0),
                            stop=(not two_pass and i == ni - 1),
                        )
                        if two_pass:
                            rhs_lo = wlo[i][:, oc * 512 : (oc + 1) * 512]
                            nc.tensor.matmul(
                                psums[oc][:],
                                lhs,
                                rhs_lo,
                                start=False,
                                stop=(i == ni - 1),
                            )
                for oc in range(noc):
                    ot = out_p.tile([128, 512], FP)
                    nc.vector.tensor_scalar(
                        ot[:], psums[oc][:], s_tiles[t][:], None, ALU.mult
                    )
                    nc.sync.dma_start(
                        out[
                            t * 128 : (t + 1) * 128,
                            o0 + oc * 512 : o0 + (oc + 1) * 512,
                        ],
                        ot[:],
                    )
    if split_waits:
        split_excess_waits(nc)
    return nc


def _shard_inputs(x, w_int, w_scales, w_zeros, n_cores):
    tok = TOK_FULL // n_cores
    xf = np.ascontiguousarray(x.reshape(TOK_FULL, D_IN).astype(np.float32))
    w8t = np.ascontiguousarray(w_int.astype(np.int8).T)  # [I, O]
    st = np.ascontiguousarray(w_scales.astype(np.float32).T)  # [G, O]
    # w_zeros is all zeros in this problem spec; fold a nonzero zeros tensor
    # into the scales path would require an extra term — assert instead.
    assert np.all(w_zeros == 0.0), "kernel assumes w_zeros == 0"
    in_maps = []
    for c in range(n_cores):
        in_maps.append(
            {
                "xs": xf[c * tok : (c + 1) * tok],
                "w8t": w8t,
                "st": st,
            }
        )
    return in_maps


_NC_CACHE = {}


def _get_nc(two_pass=True, wdt=BF):
    key = (two_pass, wdt)
    if key not in _NC_CACHE:
        _NC_CACHE[key] = build_nc(
            TOK_FULL // N_CORES, D_IN, D_OUT, oq=512, two_pass=two_pass, wdt=wdt
        )
    return _NC_CACHE[key]


def _ensure_ntff_hook():
    """This container lacks the antenv.axon_hooks shim that exposes the
    NTFF profile hook; reconstruct it from trn_boot's ctypes path."""
    import sys
    import types

    try:
        from antenv.axon_hooks import get_axon_ntff_profile_hook  # noqa: F401

        return
    except ImportError:
        pass
    hook = None
    try:
        import trn_agent_boot.trn_boot as tb

        hook = tb._ntff_profile_via_ctypes("/opt/axon/libaxon_pjrt.so")
    except Exception:
        hook = None
    mod = types.ModuleType("antenv.axon_hooks")
    mod.get_axon_ntff_profile_hook = lambda: hook
    mod.set_axon_ntff_profile_hook = lambda h: None
    import antenv

    antenv.axon_hooks = mod
    sys.modules["antenv.axon_hooks"] = mod


def kernel(x, w_int, w_scales, w_zeros, _trace=False, _two_pass=False, _wdt=F16):
    if _trace:
        _ensure_ntff_hook()
    in_maps = _shard_inputs(x, w_int, w_scales, w_zeros, N_CORES)
    nc = _get_nc(_two_pass, _wdt)
    res = bass_utils.run_bass_kernel_spmd(
        nc, in_maps, core_ids=list(range(N_CORES)), trace=_trace
    )
    tok = TOK_FULL // N_CORES
    full = np.concatenate([res.results[c]["out"] for c in range(N_CORES)], axis=0)
    out = full.reshape(B, S, D_OUT).astype(np.float32)
    if _trace:
        return out, res
    return out


# revision 15
# speedup vs baseline: 1.2722x; 1.2722x over previous
"""Trainium2 Bass kernel: per-token int8 fake-quant x  @  int4-group-dequant W^T.

Math (matches torchao-style reference):
    x_dq = per_token_quant_dequant(x)            # [B*S, I]
    w_dq = (w_int - zeros) * scales per group    # [O, I]
    out  = x_dq @ w_dq.T                         # [B*S, O]

Device factorization:
    x_dq[t, i] = s[t] * qmz[t, i]   with qmz integer in [-255, 255] (exact in fp16)
    out[t, o]  = s[t] * sum_i qmz[t, i] * w_fp16[o, i]
qmz is computed with two fused tensor_scalar passes using the +1.5*2^23
round-to-nearest-even trick; w is dequantized on device to fp16 once and
stays resident in SBUF; per-token scale is applied to PSUM on readout.

Sharding: data-parallel over tokens, 8 cores x 1024 tokens each.
"""

from contextlib import ExitStack

import numpy as np

import concourse.bass as bass
import concourse.mybir as mybir
import concourse.tile as tile
from concourse import bass_utils
from concourse import masks

FP = mybir.dt.float32
BF = mybir.dt.bfloat16
F16 = mybir.dt.float16
I8 = mybir.dt.int8
ALU = mybir.AluOpType
ACTF = mybir.ActivationFunctionType

MAGIC = 12582912.0  # 1.5 * 2**23: add/sub forces RNE round-to-integer in fp32
EPS32 = float(np.finfo(np.float32).eps)
GROUP = 32

N_CORES = 8
B, S, D_IN, D_OUT = 4, 2048, 2048, 2048
TOK_FULL = B * S

MAX_WAITS_PER_INST = 1


def split_excess_waits(nc, max_waits=MAX_WAITS_PER_INST):
    """This walrus build rejects instructions with more than one sync-wait
    command. Move excess waits onto same-engine NOPs placed immediately
    before the over-subscribed instruction — semantically identical (the
    engine performs all waits before issuing)."""
    n_split = 0
    for f in nc.m.functions:
        for bb in f.blocks:
            insts = bb.instructions
            if not any(
                i.sync_info is not None and len(i.sync_info.on_wait or []) > max_waits
                for i in insts
            ):
                continue
            new = []
            for inst in insts:
                si = inst.sync_info
                waits = list(si.on_wait) if si is not None and si.on_wait else []
                if len(waits) > max_waits:
                    keep = waits[-max_waits:]
                    rest = waits[: len(waits) - max_waits]
                    for j in range(0, len(rest), max_waits):
                        nop = mybir.InstNoOp(
                            name=f"wsplit_{inst.name}_{j}",
                            engine=inst.engine,
                            ins=[],
                            outs=[],
                            sync_info=mybir.SyncInfo(
                                on_wait=rest[j : j + max_waits], on_update=[]
                            ),
                        )
                        new.append(nop)
                        n_split += 1
                    si.on_wait = keep
                new.append(inst)
            insts[:] = new
    return n_split


def build_nc(tok, d_in, d_out, wdt=F16, split_waits=True):
    """Single-pass fp16 kernel: resident dequantized weights, fused quant."""
    nt = tok // 128
    ni = d_in // 128
    noc = d_out // 512
    assert tok % 128 == 0 and d_in % 128 == 0 and d_out % 512 == 0

    nc = bass.Bass("TRN2", target_bir_lowering=False, debug=False)
    xs = nc.dram_tensor("xs", [tok, d_in], FP, kind="ExternalInput").ap()
    w8t = nc.dram_tensor("w8t", [d_in, d_out], I8, kind="ExternalInput").ap()
    # host-expanded per-element scales, [d_in, d_out] fp32
    st = nc.dram_tensor("st", [d_in, d_out], FP, kind="ExternalInput").ap()
    out = nc.dram_tensor("out", [tok, d_out], FP, kind="ExternalOutput").ap()

    with tile.TileContext(nc) as tc, ExitStack() as ctx:
        const_pool = ctx.enter_context(tc.tile_pool(name="const", bufs=1))
        ident = const_pool.tile([128, 128], wdt, tag="ident", name="ident")
        masks.make_identity(nc, ident[:])

        stats = ctx.enter_context(tc.tile_pool(name="stats", bufs=1))
        xp = ctx.enter_context(tc.tile_pool(name="xp", bufs=3))
        qa_p = ctx.enter_context(tc.tile_pool(name="qa", bufs=2))
        qc_p = ctx.enter_context(tc.tile_pool(name="qc", bufs=2))
        qxt_p = ctx.enter_context(tc.tile_pool(name="qxt", bufs=1))
        w8_p = ctx.enter_context(tc.tile_pool(name="w8", bufs=3))
        sc_p = ctx.enter_context(tc.tile_pool(name="sc", bufs=3))
        wf_p = ctx.enter_context(tc.tile_pool(name="wf", bufs=1))
        out_p = ctx.enter_context(tc.tile_pool(name="outp", bufs=6))
        ps_mm = ctx.enter_context(tc.tile_pool(name="psmm", bufs=5, space="PSUM"))
        ps_tr = ctx.enter_context(tc.tile_pool(name="pstr", bufs=3, space="PSUM"))

        # ---- weight dequant: wf16[i] resident [128, d_out] fp16
        wf16 = []
        for i in range(ni):
            w8 = w8_p.tile([128, d_out], I8)
            nc.sync.dma_start(w8[:], w8t[i * 128 : (i + 1) * 128, :])
            sc = sc_p.tile([128, d_out], FP)
            nc.scalar.dma_start(sc[:], st[i * 128 : (i + 1) * 128, :])
            wf = wf_p.tile([128, d_out], wdt, tag=f"wf{i}", name=f"wf{i}")
            nc.vector.tensor_tensor(wf[:], w8[:], sc[:], ALU.mult)
            wf16.append(wf)

        # ---- per-token quant + transpose
        qxt = [
            qxt_p.tile([128, tok], wdt, tag=f"qxt{i}", name=f"qxt{i}")
            for i in range(ni)
        ]
        s_tiles = []
        for t in range(nt):
            xt = xp.tile([128, d_in], FP)
            nc.sync.dma_start(xt[:], xs[t * 128 : (t + 1) * 128, :])

            mn = stats.tile([128, 1], FP, tag=f"mn{t}", name=f"mn{t}")
            mx = stats.tile([128, 1], FP, tag=f"mx{t}", name=f"mx{t}")
            nc.vector.tensor_reduce(mn[:], xt[:], mybir.AxisListType.X, ALU.min)
            nc.vector.tensor_reduce(mx[:], xt[:], mybir.AxisListType.X, ALU.max)
            nc.vector.tensor_scalar(mn[:], mn[:], 0.0, None, ALU.min)
            nc.vector.tensor_scalar(mx[:], mx[:], 0.0, None, ALU.max)
            s_t = stats.tile([128, 1], FP, tag=f"s{t}", name=f"s{t}")
            nc.vector.tensor_tensor(s_t[:], mx[:], mn[:], ALU.subtract)
            nc.vector.tensor_scalar(
                s_t[:], s_t[:], float(np.float32(1.0) / np.float32(255.0)), EPS32,
                ALU.mult, ALU.max,
            )
            inv = stats.tile([128, 1], FP, tag=f"inv{t}", name=f"inv{t}")
            nc.vector.reciprocal(inv[:], s_t[:])
            # u = rne(mn*inv);  c1 = (M + 255) + u  [= 127 - zp + M]
            u = stats.tile([128, 1], FP, tag=f"u{t}", name=f"u{t}")
            nc.vector.tensor_tensor(u[:], mn[:], inv[:], ALU.mult)
            nc.vector.tensor_scalar(u[:], u[:], MAGIC, None, ALU.add)
            nc.vector.tensor_scalar(u[:], u[:], MAGIC, None, ALU.subtract)
            c1 = stats.tile([128, 1], FP, tag=f"c1{t}", name=f"c1{t}")
            nc.vector.tensor_scalar(c1[:], u[:], MAGIC + 255.0, None, ALU.add)
            s_tiles.append(s_t)

            # qa = x*inv + M  (fp32 int+M); qmz = min(qa, c1) - M  -> fp16
            qa = qa_p.tile([128, d_in], FP)
            nc.gpsimd.tensor_scalar(qa[:], xt[:], inv[:], MAGIC, ALU.mult, ALU.add)
            qc = qc_p.tile([128, d_in], wdt)
            nc.vector.tensor_scalar(qc[:], qa[:], c1[:], MAGIC, ALU.min, ALU.subtract)

            for i in range(ni):
                tr = ps_tr.tile([128, 128], wdt)
                nc.tensor.transpose(tr[:], qc[:, i * 128 : (i + 1) * 128], ident[:])
                nc.scalar.activation(
                    qxt[i][:, t * 128 : (t + 1) * 128], tr[:], ACTF.Copy
                )

        # ---- matmul: for each token tile, accumulate over i with shared lhsT
        for t in range(nt):
            psums = [
                ps_mm.tile([128, 512], FP, tag="ps", name=f"ps_t{t}_{_oc}")
                for _oc in range(noc)
            ]
            for i in range(ni):
                lhs = qxt[i][:, t * 128 : (t + 1) * 128]
                for oc in range(noc):
                    nc.tensor.matmul(
                        psums[oc][:],
                        lhs,
                        wf16[i][:, oc * 512 : (oc + 1) * 512],
                        start=(i == 0),
                        stop=(i == ni - 1),
                    )
            for oc in range(noc):
                ot = out_p.tile([128, 512], FP)
                nc.scalar.mul(ot[:], psums[oc][:], s_tiles[t][:])
                nc.gpsimd.dma_start(
                    out[t * 128 : (t + 1) * 128, oc * 512 : (oc + 1) * 512],
                    ot[:],
                )
    if split_waits:
        split_excess_waits(nc)
    return nc


def _shard_inputs(x, w_int, w_scales, w_zeros, n_cores):
    tok = TOK_FULL // n_cores
    xf = np.ascontiguousarray(x.reshape(TOK_FULL, D_IN).astype(np.float32))
    w8t = np.ascontiguousarray(w_int.astype(np.int8).T)  # [I, O]
    # per-element scale, transposed+expanded on host: st[i, o] = w_scales[o, i//32]
    st = np.ascontiguousarray(
        np.repeat(w_scales.astype(np.float32).T, GROUP, axis=0)
    )  # [I, O]
    assert np.all(w_zeros == 0.0), "kernel assumes w_zeros == 0"
    in_maps = []
    for c in range(n_cores):
        in_maps.append(
            {"xs": xf[c * tok : (c + 1) * tok], "w8t": w8t, "st": st}
        )
    return in_maps


_NC_CACHE = {}


def _get_nc(wdt=F16):
    key = wdt
    if key not in _NC_CACHE:
        _NC_CACHE[key] = build_nc(TOK_FULL // N_CORES, D_IN, D_OUT, wdt=wdt)
    return _NC_CACHE[key]


def _ensure_ntff_hook():
    """This container lacks the antenv.axon_hooks shim that exposes the
    NTFF profile hook; reconstruct it from trn_boot's ctypes path."""
    import sys
    import types

    try:
        from antenv.axon_hooks import get_axon_ntff_profile_hook  # noqa: F401

        return
    except ImportError:
        pass
    hook = None
    try:
        import trn_agent_boot.trn_boot as tb

        hook = tb._ntff_profile_via_ctypes("/opt/axon/libaxon_pjrt.so")
    except Exception:
        hook = None
    mod = types.ModuleType("antenv.axon_hooks")
    mod.get_axon_ntff_profile_hook = lambda: hook
    mod.set_axon_ntff_profile_hook = lambda h: None
    import antenv

    antenv.axon_hooks = mod
    sys.modules["antenv.axon_hooks"] = mod


def kernel(x, w_int, w_scales, w_zeros, _trace=False, _wdt=F16):
    if _trace:
        _ensure_ntff_hook()
    in_maps = _shard_inputs(x, w_int, w_scales, w_zeros, N_CORES)
    nc = _get_nc(_wdt)
    res = bass_utils.run_bass_kernel_spmd(
        nc, in_maps, core_ids=list(range(N_CORES)), trace=_trace
    )
    tok = TOK_FULL // N_CORES
    full = np.concatenate([res.results[c]["out"] for c in range(N_CORES)], axis=0)
    out = full.reshape(B, S, D_OUT).astype(np.float32)
    if _trace:
        return out, res
    return out
